# revision 30
# baseline (speedup 1.0000x reference)
"""Group whitening (decorrelated batch norm) kernel for 8 TRN2 NeuronCores.

Math (matches the reference):
  x_in = x.transpose(1,0,2,3,4).reshape(G, m)       # G=16, m = N*C*H*W
  Sigma = cov(x_in) + eps*I ; Sigma_N = Sigma / tr(Sigma)
  L = chol(Sigma_N); wm = L^-1 (lower-tri); out = wm @ x_in

Distribution: data-parallel over m. Core c owns n in {2c, 2c+1} (m is
n-major so this is a contiguous m-shard). Each core computes a partial
Gram matrix S = X X^T and row-sums s over its shard, the tiny [16,17]
stats are AllReduce'd, every core solves the same 16x16 factorization
on-device, and applies wm to its local shard.

On-chip layout: the shard lives residently in SBUF as bf16 [128, T]
with partition p = b*16+g (b = 8 column-blocks of the shard, g =
group):
  - the apply is ONE matmul per column chunk with a block-diagonal
    stationary [128,128] = diag(wm^T x 8) (8 m-columns per PE column),
  - the Gram runs over xbar-DMA-transposed tiles (t on partitions),
    batched into 8 big DMA_TRANSPOSE instructions on the ACT HWDGE ring
    (loads use the SP ring, so the two pipelines don't queue behind
    each other), accumulating all cross-block products in PSUM
    [128,128]; the wanted block-diagonal 16x16 blocks are extracted at
    the end with a mask and a stacked-identity reduction matmul.
  - the 16x16 solve is an all-DVE LDL^T factorization on partition 0
    (sqrt-free, so no per-step DVE<->ACT ping-pong), finished by one
    vectorized Sqrt: wm = D^-1/2 (Lunit)^-1.
"""

import os
import numpy as np

EPS = 1e-5

# Full problem constants (hardcoded; kernel.py must be self-contained).
N_FULL, G, C, H, W = 16, 16, 64, 56, 56
CHW = C * H * W                      # 200704
N_CORES = 8
NL = N_FULL // N_CORES               # 2 n's per core
NB = 8                               # column blocks per core -> 128 partitions
P = NB * G                           # 128
M_TOT = N_FULL * CHW                 # 3,211,264 (global m)


def build_graph(nc, tc, in_ap, out_ap, *, nl, chw, n_cores):
    """Emit the SPMD program for one core (all cores run the same graph)."""
    import concourse.bass as bass
    import concourse.mybir as mybir

    import ml_dtypes
    ml_bf16 = ml_dtypes.bfloat16

    f32 = mybir.dt.float32
    bf16 = mybir.dt.bfloat16
    AX = mybir.AxisListType.X
    ALU = mybir.AluOpType
    ACTF = mybir.ActivationFunctionType

    Q = NB                           # row-eighths: all 8 blocks per n
    T = nl * chw // NB               # resident free size per partition
    TH = T // nl                     # free-range per n (n maps to free halves)
    CH = 3584 if TH % 3584 == 0 else TH        # load/store chunk
    TB = CH if CH % 128 == 0 else 128   # transpose batch = one load chunk
    MM = 512                         # apply matmul free dim (PSUM bank)
    assert TH % CH == 0 and T % TB == 0 and TB % 128 == 0 and CH % MM == 0
    n_ch = T // CH
    n_tb = T // TB
    nt = TB // 128                   # 128-wide tiles per transpose batch
    m_tot = n_cores * nl * chw

    v = nc.vector
    s = nc.scalar

    # ---- constants baked into the NEFF ----
    # partition p = g*NB + q (g-outer): g(p) = p // NB, q(p) = p % NB
    gp = np.arange(P) // NB
    qp = np.arange(P) % NB
    e_np = (gp[:, None] == np.arange(G)[None, :]).astype(np.float32)
    mask_np = (qp[:, None] == qp[None, :]).astype(np.float32)
    i16_np = np.eye(G, dtype=np.float32).reshape(1, G * G)
    epsi_np = (EPS * np.eye(G, dtype=np.float32)).reshape(1, G * G)
    et_np = e_np.T.astype(ml_bf16)                      # [G, P] selector
    maskbd_np = mask_np.astype(ml_bf16)                 # same-q mask, bf16

    e_dr = nc.inline_tensor(e_np, name="const_e")
    mask_dr = nc.inline_tensor(mask_np, name="const_mask")
    i16_dr = nc.inline_tensor(i16_np, name="const_i16")
    epsi_dr = nc.inline_tensor(epsi_np, name="const_epsi")
    et_dr = nc.inline_tensor(et_np, name="const_et")
    maskbd_dr = nc.inline_tensor(maskbd_np, name="const_maskbd")

    with (
        tc.tile_pool(name="consts", bufs=1) as cpool,
        tc.tile_pool(name="resident", bufs=1) as rpool,
        tc.tile_pool(name="stage_in", bufs=3) as sin_pool,
        tc.tile_pool(name="tt", bufs=2) as tt_pool,
        tc.tile_pool(name="stage_out", bufs=2) as sout_pool,
        tc.tile_pool(name="small", bufs=1) as spool,
        tc.tile_pool(name="psum_acc", bufs=1, space="PSUM") as pacc,
        tc.tile_pool(name="psum_apply", bufs=4, space="PSUM") as papp,
        tc.tile_pool(name="dram", bufs=1, space="DRAM") as dpool,
    ):
        e_sb = cpool.tile([P, G], f32, tag="e")
        mask_sb = cpool.tile([P, P], f32, tag="mask")
        i16_sb = cpool.tile([1, G * G], f32, tag="i16")
        epsi_sb = cpool.tile([1, G * G], f32, tag="epsi")
        et_sb = cpool.tile([G, P], bf16, tag="et")
        maskbd_sb = cpool.tile([P, P], bf16, tag="maskbd")
        nc.sync.dma_start(e_sb[:], e_dr.ap())
        nc.sync.dma_start(mask_sb[:], mask_dr.ap())
        nc.sync.dma_start(i16_sb[:], i16_dr.ap())
        nc.sync.dma_start(epsi_sb[:], epsi_dr.ap())
        nc.sync.dma_start(et_sb[:], et_dr.ap())
        nc.sync.dma_start(maskbd_sb[:], maskbd_dr.ap())

        xres = rpool.tile([P, T], bf16, tag="xres")
        sums_part = spool.tile([P, n_ch], f32, tag="sums_part")

        # DRAM views: [nl, G, chw] -> [nl, 8, G, chw/8]-shaped AP.  SBUF
        # partition p = b*16+g where b indexes the 8 row-EIGHTHS of a row;
        # n maps to the free-axis halves of the resident tile.  One load is
        # then a single full-128-partition DMA (3-dim source), which sprays
        # all 16 SDMA engines (~6x the ring throughput of partial DMAs).
        # g-outer descriptor order: consecutive DMA descriptors step the
        # small q-stride (chw/8 elems), keeping them address-local — measured
        # ~300GB/s vs ~100GB/s for q-outer.  The partition layout is
        # unchanged (p = q*16+g): the DMA balancer splits the SBUF side's
        # [128, CH] to match (g, q, t) at lowering, after dep tracking.
        xv = in_ap.rearrange("n g (q t) -> n g q t", q=Q)
        ov = out_ap.rearrange("n g (q t) -> n g q t", q=Q)

        # ---- phases 1+2 interleaved: load f32 (both HWDGE rings), cast
        # bf16 + row sums; emit each batched xbar transpose + Gram matmuls
        # as soon as its resident region is covered, so the transposes/Gram
        # overlap the load stream. ----
        gram_ps = pacc.tile([P, P], f32, tag="gram")
        tpose_done = 0

        def emit_transposes(covered):
            nonlocal tpose_done
            while tpose_done < n_tb and (tpose_done + 1) * TB <= covered:
                b = tpose_done
                ttile = tt_pool.tile([P, nt, 128], bf16, tag="tt")
                nc.scalar.dma_start_transpose(
                    ttile[:], xres[:, b * TB:(b + 1) * TB])
                for j in range(nt):
                    k = b * nt + j
                    sl = ttile[:, j, :]
                    nc.tensor.matmul(
                        gram_ps[:], lhsT=sl, rhs=sl,
                        start=(k == 0), stop=(k == n_tb * nt - 1),
                    )
                tpose_done += 1

        for kg in range(n_ch):
            n, k = kg // (TH // CH), kg % (TH // CH)
            lo = n * TH + k * CH
            st = sin_pool.tile([P, CH], f32, tag="stin")
            nc.sync.dma_start(st[:], xv[n, :, :, k * CH:(k + 1) * CH])
            if kg % 2 == 0:
                v.tensor_scalar(
                    xres[:, lo:lo + CH], st[:], 1.0, None, ALU.mult,
                    ALU.add, accum_out=sums_part[:, kg:kg + 1],
                )
            else:
                s.activation(
                    xres[:, lo:lo + CH], st[:], ACTF.Copy,
                    accum_out=sums_part[:, kg:kg + 1],
                )
            emit_transposes(lo + CH)

        # ---- phase 3: extract block-diagonal S and sums ----
        p_sb = spool.tile([P, P + 4], f32, tag="p_sb")
        v.tensor_tensor(p_sb[:, 0:P], gram_ps[:], mask_sb[:], op=ALU.mult)
        v.tensor_reduce(p_sb[:, P:P + 1], sums_part[:], AX, ALU.add)

        q_ps = pacc.tile([G, P + 4], f32, tag="q_ps")
        nc.tensor.matmul(
            q_ps[:, 0:P + 1], lhsT=e_sb[:], rhs=p_sb[:, 0:P + 1],
            start=True, stop=True,
        )
        q_sb = spool.tile([G, P + 4], f32, tag="q_sb")
        v.tensor_copy(q_sb[:, 0:P + 1], q_ps[:, 0:P + 1])
        # fold the NB same-q lanes: S[g1, go] = sum_q Q[g1, go*8 + q]
        q3 = q_sb[:, 0:P].rearrange("p (go q) -> p go q", q=NB)
        v.tensor_tensor(q3[:, 0:G, 0:4], q3[:, 0:G, 0:4], q3[:, 0:G, 4:8],
                        op=ALU.add)
        v.tensor_tensor(q3[:, 0:G, 0:2], q3[:, 0:G, 0:2], q3[:, 0:G, 2:4],
                        op=ALU.add)
        v.tensor_tensor(q3[:, 0:G, 0:1], q3[:, 0:G, 0:1], q3[:, 0:G, 1:2],
                        op=ALU.add)

        ar_sb = spool.tile([G, G + 1], f32, tag="ar_sb")
        v.tensor_copy(ar_sb[:, 0:G], q_sb[:, 0:P:NB])
        v.tensor_copy(ar_sb[:, G:G + 1], q_sb[:, P:P + 1])

        # ---- phase 4: AllReduce the [16,17] stats ----
        cc_in = dpool.tile([G, G + 1], f32, tag="cc_in")
        cc_out = dpool.tile([G, G + 1], f32, tag="cc_out")
        nc.sync.dma_start(cc_in[:], ar_sb[:])
        nc.gpsimd.collective_compute(
            "AllReduce", mybir.AluOpType.add,
            replica_groups=[list(range(n_cores))],
            ins=[cc_in.opt()],
            outs=[cc_out.opt()],
        )
        sp = spool.tile([1, G * (G + 1)], f32, tag="sp")   # [1, 272]
        nc.sync.dma_start(sp[:], cc_out[:])

        # ---- phase 5: Sigma -> LDL^T -> wm = D^-1/2 Lunit^-1, partition 0 --
        # sp flat layout: S[g1,g2] at 17*g1+g2, s[g1] at 17*g1+16
        a_t = spool.tile([1, G * G], f32, tag="a_t")
        l_t = spool.tile([1, G * G], f32, tag="l_t")
        w_t = spool.tile([1, G * G], f32, tag="w_t")
        tmp_t = spool.tile([1, G * G], f32, tag="tmp_t")
        mean_t = spool.tile([1, G], f32, tag="mean_t")
        rd_t = spool.tile([1, G], f32, tag="rd_t")
        sd_t = spool.tile([1, G], f32, tag="sd_t")
        rsd_t = spool.tile([1, G], f32, tag="rsd_t")
        sc_t = spool.tile([1, 4], f32, tag="sc_t")

        sp3 = sp[:].rearrange("p (a b) -> p a b", b=G + 1)
        a3 = a_t[:].rearrange("p (a b) -> p a b", b=G)
        l3 = l_t[:].rearrange("p (a b) -> p a b", b=G)
        w3 = w_t[:].rearrange("p (a b) -> p a b", b=G)
        t3 = tmp_t[:].rearrange("p (a b) -> p a b", b=G)

        minv = 1.0 / float(m_tot)
        # mean = s/m ; A = S/m - mean mean^T + eps I
        v.tensor_scalar(
            mean_t[:].rearrange("p (g o) -> p g o", o=1),
            sp3[:, :, G:G + 1], minv, None, ALU.mult,
        )
        v.tensor_scalar(a3, sp3[:, :, 0:G], minv, None, ALU.mult)
        bc_i = mean_t[:].to_broadcast([1, G, G])          # mean[i] over j
        bc_j = bc_i.rearrange("p i j -> p j i")           # mean[j] over i
        v.tensor_tensor(t3, bc_i, bc_j, op=ALU.mult)
        v.tensor_tensor(a_t[:], a_t[:], tmp_t[:], op=ALU.subtract)
        v.tensor_tensor(a_t[:], a_t[:], epsi_sb[:], op=ALU.add)
        # trace-normalize: A *= 1/tr(A)
        v.tensor_reduce(sc_t[:, 0:1], a_t[:, 0:G * G:G + 1], AX, ALU.add)
        v.reciprocal(sc_t[:, 1:2], sc_t[:, 0:1])
        v.tensor_scalar(a_t[:], a_t[:], sc_t[:, 1:2], None, ALU.mult)

        # LDL^T: A = Lunit D Lunit^T, in-place downdates, all on DVE.
        for j in range(G):
            dj = a_t[:, j * (G + 1):j * (G + 1) + 1]
            v.reciprocal(rd_t[:, j:j + 1], dj)
            # Lunit[i,j] = A[i,j] / d_j for i = j..15 (strided over i)
            v.tensor_scalar(
                l_t[:, j * (G + 1):G * G:G],
                a_t[:, j * (G + 1):G * G:G],
                rd_t[:, j:j + 1], None, ALU.mult,
            )
            if j < G - 1:
                r = G - 1 - j
                asub = a3[:, j + 1:G, j + 1:G]
                li = l3[:, j + 1:G, j:j + 1].to_broadcast([1, r, r])
                ak = a3[:, j + 1:G, j:j + 1].rearrange("p i o -> p o i") \
                    .to_broadcast([1, r, r])
                v.tensor_tensor(t3[:, 0:r, 0:r], li, ak, op=ALU.mult)
                v.tensor_tensor(asub, asub, t3[:, 0:r, 0:r], op=ALU.subtract)

        # W = Lunit^-1 (unit lower): W=I; W[i,:] -= L[i,j] W[j,:]
        v.tensor_copy(w_t[:], i16_sb[:])
        for j in range(G - 1):
            r = G - 1 - j
            wsub = w3[:, j + 1:G, 0:j + 1]
            li = l3[:, j + 1:G, j:j + 1].to_broadcast([1, r, j + 1])
            wrow = w3[:, j:j + 1, 0:j + 1].to_broadcast([1, r, j + 1])
            v.tensor_tensor(t3[:, 0:r, 0:j + 1], li, wrow, op=ALU.mult)
            v.tensor_tensor(wsub, wsub, t3[:, 0:r, 0:j + 1], op=ALU.subtract)

        # wm = D^-1/2 W, written TRANSPOSED (wmT[g, go] = wm[go, g]) and cast
        # to bf16 in the same op
        s.activation(sd_t[:], a_t[:, 0:G * G:G + 1], ACTF.Sqrt)
        v.reciprocal(rsd_t[:], sd_t[:])
        wmbf = spool.tile([1, G * G], bf16, tag="wmbf")
        wmT3 = wmbf[:].rearrange("p (g go) -> p g go", go=G)
        v.tensor_tensor(
            wmT3,
            w3.rearrange("p go g -> p g go"),
            rsd_t[:].rearrange("p (go o) -> p o go", o=1).to_broadcast([1, G, G]),
            op=ALU.mult,
        )

        # ---- phase 6: apply stationary BD[p1,p2] = wm[go(p2), g(p1)] for
        # q(p1)==q(p2).  Build: wmT -> DRAM -> [16,16] partition-spread,
        # broadcast-expand to [16,128], one selector matmul, masked evac. ----
        wm_dr = dpool.tile([G * G], bf16, tag="wm_dr")
        nc.scalar.dma_start(wm_dr[:], wmbf[:])
        wmt_sb = spool.tile([G, G], bf16, tag="wmt_sb")
        nc.scalar.dma_start(wmt_sb[:], wm_dr[:].rearrange("(g go) -> g go", g=G))
        wmx = spool.tile([G, P], bf16, tag="wmx")
        v.tensor_copy(
            wmx[:].rearrange("p (go q) -> p go q", q=NB),
            wmt_sb[:].rearrange("p (go o) -> p go o", o=1).to_broadcast([G, G, NB]),
        )
        bd_ps = pacc.tile([P, P], f32, tag="bd_ps")
        nc.tensor.matmul(bd_ps[:], lhsT=et_sb[:], rhs=wmx[:],
                         start=True, stop=True)
        bd = cpool.tile([P, P], bf16, tag="bd")
        v.tensor_tensor(bd[:], bd_ps[:], maskbd_sb[:], op=ALU.mult)

        # ---- phase 7: apply out = wm @ x and store (both rings) ----
        for kg in range(n_ch):
            n, k = kg // (TH // CH), kg % (TH // CH)
            so = sout_pool.tile([P, CH], f32, tag="so")
            for i in range(CH // MM):
                aps = papp.tile([P, MM], f32, tag="aps")
                lo = n * TH + k * CH + i * MM
                nc.tensor.matmul(
                    aps[:], lhsT=bd[:], rhs=xres[:, lo:lo + MM],
                    start=True, stop=True,
                )
                if i % 2 == 0:
                    v.tensor_copy(so[:, i * MM:(i + 1) * MM], aps[:])
                else:
                    s.copy(so[:, i * MM:(i + 1) * MM], aps[:])
            ring = nc.sync if kg % 2 == 0 else nc.scalar
            ring.dma_start(ov[n, :, :, k * CH:(k + 1) * CH], so[:])


def make_nc(*, nl=NL, chw=CHW, n_cores=N_CORES):
    import concourse.bacc as bacc
    import concourse.mybir as mybir
    import concourse.tile as tile

    nc = bacc.Bacc(
        "TRN2",
        target_bir_lowering=False,
        debug=False,
        enable_asserts=False,
        num_devices=n_cores,
    )
    x_dr = nc.dram_tensor("x", [nl, G, chw], mybir.dt.float32,
                          kind="ExternalInput")
    out_dr = nc.dram_tensor("out", [nl, G, chw], mybir.dt.float32,
                            kind="ExternalOutput")
    with tile.TileContext(nc) as tc:
        build_graph(nc, tc, x_dr.ap(), out_dr.ap(),
                    nl=nl, chw=chw, n_cores=n_cores)
    nc.compile()
    return nc


def kernel(x: np.ndarray) -> np.ndarray:
    from concourse.bass_utils import run_bass_kernel_spmd

    assert x.shape == (N_FULL, G, C, H, W) and x.dtype == np.float32
    xr = np.ascontiguousarray(x.reshape(N_FULL, G, CHW))
    in_maps = [
        {"x": np.ascontiguousarray(xr[c * NL:(c + 1) * NL])}
        for c in range(N_CORES)
    ]
    nc = make_nc()
    trace = bool(int(os.environ.get("KERNEL_TRACE", "0")))
    res = run_bass_kernel_spmd(
        nc, in_maps, core_ids=list(range(N_CORES)), trace=trace,
    )
    if trace and res.exec_time_ns is not None:
        print(f"HW exec time: {res.exec_time_ns} ns")
    out = np.concatenate([res.results[c]["out"] for c in range(N_CORES)], axis=0)
    return np.ascontiguousarray(out.reshape(N_FULL, G, C, H, W))


# revision 33
# speedup vs baseline: 1.0538x; 1.0538x over previous
"""Group whitening (decorrelated batch norm) kernel for 8 TRN2 NeuronCores.

Math (matches the reference):
  x_in = x.transpose(1,0,2,3,4).reshape(G, m)       # G=16, m = N*C*H*W
  Sigma = cov(x_in) + eps*I ; Sigma_N = Sigma / tr(Sigma)
  L = chol(Sigma_N); wm = L^-1 (lower-tri); out = wm @ x_in

Distribution: data-parallel over m. Core c owns n in {2c, 2c+1} (m is
n-major so this is a contiguous m-shard). Each core computes a partial
Gram matrix S = X X^T and row-sums s over its shard, the tiny [16,17]
stats are AllReduce'd, every core solves the same 16x16 factorization
on-device, and applies wm to its local shard.

On-chip layout: the shard lives residently in SBUF as bf16 [128, T]
with partition p = b*16+g (b = 8 column-blocks of the shard, g =
group):
  - the apply is ONE matmul per column chunk with a block-diagonal
    stationary [128,128] = diag(wm^T x 8) (8 m-columns per PE column),
  - the Gram runs over xbar-DMA-transposed tiles (t on partitions),
    batched into 8 big DMA_TRANSPOSE instructions on the ACT HWDGE ring
    (loads use the SP ring, so the two pipelines don't queue behind
    each other), accumulating all cross-block products in PSUM
    [128,128]; the wanted block-diagonal 16x16 blocks are extracted at
    the end with a mask and a stacked-identity reduction matmul.
  - the 16x16 solve is an all-DVE LDL^T factorization on partition 0
    (sqrt-free, so no per-step DVE<->ACT ping-pong), finished by one
    vectorized Sqrt: wm = D^-1/2 (Lunit)^-1.
"""

import os
import numpy as np

EPS = 1e-5

# Full problem constants (hardcoded; kernel.py must be self-contained).
N_FULL, G, C, H, W = 16, 16, 64, 56, 56
CHW = C * H * W                      # 200704
N_CORES = 8
NL = N_FULL // N_CORES               # 2 n's per core
NB = 8                               # column blocks per core -> 128 partitions
P = NB * G                           # 128
M_TOT = N_FULL * CHW                 # 3,211,264 (global m)


def build_graph(nc, tc, in_ap, out_ap, *, nl, chw, n_cores):
    """Emit the SPMD program for one core (all cores run the same graph)."""
    import concourse.bass as bass
    import concourse.mybir as mybir

    import ml_dtypes
    ml_bf16 = ml_dtypes.bfloat16

    f32 = mybir.dt.float32
    bf16 = mybir.dt.bfloat16
    AX = mybir.AxisListType.X
    ALU = mybir.AluOpType
    ACTF = mybir.ActivationFunctionType

    Q = NB                           # row-eighths: all 8 blocks per n
    T = nl * chw // NB               # resident free size per partition
    TH = T // nl                     # free-range per n (n maps to free halves)
    CH = 3584 if TH % 3584 == 0 else TH        # load/store chunk
    TB = CH if CH % 128 == 0 else 128   # transpose batch = one load chunk
    MM = 512                         # apply matmul free dim (PSUM bank)
    assert TH % CH == 0 and T % TB == 0 and TB % 128 == 0 and CH % MM == 0
    n_ch = T // CH
    n_tb = T // TB
    nt = TB // 128                   # 128-wide tiles per transpose batch
    m_tot = n_cores * nl * chw

    v = nc.vector
    s = nc.scalar

    # ---- constants baked into the NEFF ----
    # partition p = g*NB + q (g-outer): g(p) = p // NB, q(p) = p % NB
    gp = np.arange(P) // NB
    qp = np.arange(P) % NB
    e_np = (gp[:, None] == np.arange(G)[None, :]).astype(np.float32)
    mask_np = (qp[:, None] == qp[None, :]).astype(np.float32)
    i16_np = np.eye(G, dtype=np.float32).reshape(1, G * G)
    epsi_np = (EPS * np.eye(G, dtype=np.float32)).reshape(1, G * G)
    et_np = e_np.T.astype(ml_bf16)                      # [G, P] selector
    maskbd_np = mask_np.astype(ml_bf16)                 # same-q mask, bf16

    e_dr = nc.inline_tensor(e_np, name="const_e")
    mask_dr = nc.inline_tensor(mask_np, name="const_mask")
    i16_dr = nc.inline_tensor(i16_np, name="const_i16")
    epsi_dr = nc.inline_tensor(epsi_np, name="const_epsi")
    et_dr = nc.inline_tensor(et_np, name="const_et")
    maskbd_dr = nc.inline_tensor(maskbd_np, name="const_maskbd")

    with (
        tc.tile_pool(name="consts", bufs=1) as cpool,
        tc.tile_pool(name="resident", bufs=1) as rpool,
        tc.tile_pool(name="stage_in", bufs=3) as sin_pool,
        tc.tile_pool(name="tt", bufs=2) as tt_pool,
        tc.tile_pool(name="stage_out", bufs=2) as sout_pool,
        tc.tile_pool(name="small", bufs=1) as spool,
        tc.tile_pool(name="psum_acc", bufs=1, space="PSUM") as pacc,
        tc.tile_pool(name="psum_apply", bufs=4, space="PSUM") as papp,
        tc.tile_pool(name="dram", bufs=1, space="DRAM") as dpool,
    ):
        e_sb = cpool.tile([P, G], f32, tag="e")
        mask_sb = cpool.tile([P, P], f32, tag="mask")
        i16_sb = cpool.tile([1, G * G], f32, tag="i16")
        epsi_sb = cpool.tile([1, G * G], f32, tag="epsi")
        et_sb = cpool.tile([G, P], bf16, tag="et")
        maskbd_sb = cpool.tile([P, P], bf16, tag="maskbd")
        nc.sync.dma_start(e_sb[:], e_dr.ap())
        nc.sync.dma_start(mask_sb[:], mask_dr.ap())
        nc.sync.dma_start(i16_sb[:], i16_dr.ap())
        nc.sync.dma_start(epsi_sb[:], epsi_dr.ap())
        nc.sync.dma_start(et_sb[:], et_dr.ap())
        nc.sync.dma_start(maskbd_sb[:], maskbd_dr.ap())

        xres = rpool.tile([P, T], bf16, tag="xres")
        sums_part = spool.tile([P, n_ch], f32, tag="sums_part")

        # DRAM views: [nl, G, chw] -> [nl, 8, G, chw/8]-shaped AP.  SBUF
        # partition p = b*16+g where b indexes the 8 row-EIGHTHS of a row;
        # n maps to the free-axis halves of the resident tile.  One load is
        # then a single full-128-partition DMA (3-dim source), which sprays
        # all 16 SDMA engines (~6x the ring throughput of partial DMAs).
        # g-outer descriptor order: consecutive DMA descriptors step the
        # small q-stride (chw/8 elems), keeping them address-local — measured
        # ~300GB/s vs ~100GB/s for q-outer.  The partition layout is
        # unchanged (p = q*16+g): the DMA balancer splits the SBUF side's
        # [128, CH] to match (g, q, t) at lowering, after dep tracking.
        xv = in_ap.rearrange("n g (q t) -> n g q t", q=Q)
        ov = out_ap.rearrange("n g (q t) -> n g q t", q=Q)

        # ---- phases 1+2 interleaved: load f32 (both HWDGE rings), cast
        # bf16 + row sums; emit each batched xbar transpose + Gram matmuls
        # as soon as its resident region is covered, so the transposes/Gram
        # overlap the load stream. ----
        gram_ps = pacc.tile([P, P], f32, tag="gram")
        tpose_done = 0

        def emit_transposes(covered):
            nonlocal tpose_done
            while tpose_done < n_tb and (tpose_done + 1) * TB <= covered:
                b = tpose_done
                ttile = tt_pool.tile([P, nt, 128], bf16, tag="tt")
                nc.sync.dma_start_transpose(
                    ttile[:], xres[:, b * TB:(b + 1) * TB])
                for j in range(nt):
                    k = b * nt + j
                    sl = ttile[:, j, :]
                    nc.tensor.matmul(
                        gram_ps[:], lhsT=sl, rhs=sl,
                        start=(k == 0), stop=(k == n_tb * nt - 1),
                    )
                tpose_done += 1

        for kg in range(n_ch):
            n, k = kg // (TH // CH), kg % (TH // CH)
            lo = n * TH + k * CH
            st = sin_pool.tile([P, CH], f32, tag="stin")
            nc.gpsimd.dma_start(st[:], xv[n, :, :, k * CH:(k + 1) * CH])
            if kg % 2 == 0:
                v.tensor_scalar(
                    xres[:, lo:lo + CH], st[:], 1.0, None, ALU.mult,
                    ALU.add, accum_out=sums_part[:, kg:kg + 1],
                )
            else:
                s.activation(
                    xres[:, lo:lo + CH], st[:], ACTF.Copy,
                    accum_out=sums_part[:, kg:kg + 1],
                )
            emit_transposes(lo + CH)

        # ---- phase 3: extract block-diagonal S and sums ----
        p_sb = spool.tile([P, P + 4], f32, tag="p_sb")
        v.tensor_tensor(p_sb[:, 0:P], gram_ps[:], mask_sb[:], op=ALU.mult)
        v.tensor_reduce(p_sb[:, P:P + 1], sums_part[:], AX, ALU.add)

        q_ps = pacc.tile([G, P + 4], f32, tag="q_ps")
        nc.tensor.matmul(
            q_ps[:, 0:P + 1], lhsT=e_sb[:], rhs=p_sb[:, 0:P + 1],
            start=True, stop=True,
        )
        q_sb = spool.tile([G, P + 4], f32, tag="q_sb")
        v.tensor_copy(q_sb[:, 0:P + 1], q_ps[:, 0:P + 1])
        # fold the NB same-q lanes: S[g1, go] = sum_q Q[g1, go*8 + q]
        q3 = q_sb[:, 0:P].rearrange("p (go q) -> p go q", q=NB)
        v.tensor_tensor(q3[:, 0:G, 0:4], q3[:, 0:G, 0:4], q3[:, 0:G, 4:8],
                        op=ALU.add)
        v.tensor_tensor(q3[:, 0:G, 0:2], q3[:, 0:G, 0:2], q3[:, 0:G, 2:4],
                        op=ALU.add)
        v.tensor_tensor(q3[:, 0:G, 0:1], q3[:, 0:G, 0:1], q3[:, 0:G, 1:2],
                        op=ALU.add)

        ar_sb = spool.tile([G, G + 1], f32, tag="ar_sb")
        v.tensor_copy(ar_sb[:, 0:G], q_sb[:, 0:P:NB])
        v.tensor_copy(ar_sb[:, G:G + 1], q_sb[:, P:P + 1])

        # ---- phase 4: AllReduce the [16,17] stats ----
        cc_in = dpool.tile([G, G + 1], f32, tag="cc_in")
        cc_out = dpool.tile([G, G + 1], f32, tag="cc_out")
        nc.sync.dma_start(cc_in[:], ar_sb[:])
        nc.gpsimd.collective_compute(
            "AllReduce", mybir.AluOpType.add,
            replica_groups=[list(range(n_cores))],
            ins=[cc_in.opt()],
            outs=[cc_out.opt()],
        )
        sp = spool.tile([1, G * (G + 1)], f32, tag="sp")   # [1, 272]
        nc.sync.dma_start(sp[:], cc_out[:])

        # ---- phase 5: Sigma -> LDL^T -> wm = D^-1/2 Lunit^-1, partition 0 --
        # sp flat layout: S[g1,g2] at 17*g1+g2, s[g1] at 17*g1+16
        a_t = spool.tile([1, G * G], f32, tag="a_t")
        l_t = spool.tile([1, G * G], f32, tag="l_t")
        w_t = spool.tile([1, G * G], f32, tag="w_t")
        tmp_t = spool.tile([1, G * G], f32, tag="tmp_t")
        mean_t = spool.tile([1, G], f32, tag="mean_t")
        rd_t = spool.tile([1, G], f32, tag="rd_t")
        sd_t = spool.tile([1, G], f32, tag="sd_t")
        rsd_t = spool.tile([1, G], f32, tag="rsd_t")
        sc_t = spool.tile([1, 4], f32, tag="sc_t")

        sp3 = sp[:].rearrange("p (a b) -> p a b", b=G + 1)
        a3 = a_t[:].rearrange("p (a b) -> p a b", b=G)
        l3 = l_t[:].rearrange("p (a b) -> p a b", b=G)
        w3 = w_t[:].rearrange("p (a b) -> p a b", b=G)
        t3 = tmp_t[:].rearrange("p (a b) -> p a b", b=G)

        minv = 1.0 / float(m_tot)
        # mean = s/m ; A = S/m - mean mean^T + eps I
        v.tensor_scalar(
            mean_t[:].rearrange("p (g o) -> p g o", o=1),
            sp3[:, :, G:G + 1], minv, None, ALU.mult,
        )
        v.tensor_scalar(a3, sp3[:, :, 0:G], minv, None, ALU.mult)
        bc_i = mean_t[:].to_broadcast([1, G, G])          # mean[i] over j
        bc_j = bc_i.rearrange("p i j -> p j i")           # mean[j] over i
        v.tensor_tensor(t3, bc_i, bc_j, op=ALU.mult)
        v.tensor_tensor(a_t[:], a_t[:], tmp_t[:], op=ALU.subtract)
        v.tensor_tensor(a_t[:], a_t[:], epsi_sb[:], op=ALU.add)
        # trace-normalize: A *= 1/tr(A)
        v.tensor_reduce(sc_t[:, 0:1], a_t[:, 0:G * G:G + 1], AX, ALU.add)
        v.reciprocal(sc_t[:, 1:2], sc_t[:, 0:1])
        v.tensor_scalar(a_t[:], a_t[:], sc_t[:, 1:2], None, ALU.mult)

        # LDL^T: A = Lunit D Lunit^T, in-place downdates, all on DVE.
        for j in range(G):
            dj = a_t[:, j * (G + 1):j * (G + 1) + 1]
            v.reciprocal(rd_t[:, j:j + 1], dj)
            # Lunit[i,j] = A[i,j] / d_j for i = j..15 (strided over i)
            v.tensor_scalar(
                l_t[:, j * (G + 1):G * G:G],
                a_t[:, j * (G + 1):G * G:G],
                rd_t[:, j:j + 1], None, ALU.mult,
            )
            if j < G - 1:
                r = G - 1 - j
                asub = a3[:, j + 1:G, j + 1:G]
                li = l3[:, j + 1:G, j:j + 1].to_broadcast([1, r, r])
                ak = a3[:, j + 1:G, j:j + 1].rearrange("p i o -> p o i") \
                    .to_broadcast([1, r, r])
                v.tensor_tensor(t3[:, 0:r, 0:r], li, ak, op=ALU.mult)
                v.tensor_tensor(asub, asub, t3[:, 0:r, 0:r], op=ALU.subtract)

        # W = Lunit^-1 (unit lower): W=I; W[i,:] -= L[i,j] W[j,:]
        v.tensor_copy(w_t[:], i16_sb[:])
        for j in range(G - 1):
            r = G - 1 - j
            wsub = w3[:, j + 1:G, 0:j + 1]
            li = l3[:, j + 1:G, j:j + 1].to_broadcast([1, r, j + 1])
            wrow = w3[:, j:j + 1, 0:j + 1].to_broadcast([1, r, j + 1])
            v.tensor_tensor(t3[:, 0:r, 0:j + 1], li, wrow, op=ALU.mult)
            v.tensor_tensor(wsub, wsub, t3[:, 0:r, 0:j + 1], op=ALU.subtract)

        # wm = D^-1/2 W, written TRANSPOSED (wmT[g, go] = wm[go, g]) and cast
        # to bf16 in the same op
        s.activation(sd_t[:], a_t[:, 0:G * G:G + 1], ACTF.Sqrt)
        v.reciprocal(rsd_t[:], sd_t[:])
        wmbf = spool.tile([1, G * G], bf16, tag="wmbf")
        wmT3 = wmbf[:].rearrange("p (g go) -> p g go", go=G)
        v.tensor_tensor(
            wmT3,
            w3.rearrange("p go g -> p g go"),
            rsd_t[:].rearrange("p (go o) -> p o go", o=1).to_broadcast([1, G, G]),
            op=ALU.mult,
        )

        # ---- phase 6: apply stationary BD[p1,p2] = wm[go(p2), g(p1)] for
        # q(p1)==q(p2).  Build: wmT -> DRAM -> [16,16] partition-spread,
        # broadcast-expand to [16,128], one selector matmul, masked evac. ----
        wm_dr = dpool.tile([G * G], bf16, tag="wm_dr")
        nc.scalar.dma_start(wm_dr[:], wmbf[:])
        wmt_sb = spool.tile([G, G], bf16, tag="wmt_sb")
        nc.scalar.dma_start(wmt_sb[:], wm_dr[:].rearrange("(g go) -> g go", g=G))
        wmx = spool.tile([G, P], bf16, tag="wmx")
        v.tensor_copy(
            wmx[:].rearrange("p (go q) -> p go q", q=NB),
            wmt_sb[:].rearrange("p (go o) -> p go o", o=1).to_broadcast([G, G, NB]),
        )
        bd_ps = pacc.tile([P, P], f32, tag="bd_ps")
        nc.tensor.matmul(bd_ps[:], lhsT=et_sb[:], rhs=wmx[:],
                         start=True, stop=True)
        bd = cpool.tile([P, P], bf16, tag="bd")
        v.tensor_tensor(bd[:], bd_ps[:], maskbd_sb[:], op=ALU.mult)

        # ---- phase 7: apply out = wm @ x and store (both rings) ----
        for kg in range(n_ch):
            n, k = kg // (TH // CH), kg % (TH // CH)
            so = sout_pool.tile([P, CH], f32, tag="so")
            for i in range(CH // MM):
                aps = papp.tile([P, MM], f32, tag="aps")
                lo = n * TH + k * CH + i * MM
                nc.tensor.matmul(
                    aps[:], lhsT=bd[:], rhs=xres[:, lo:lo + MM],
                    start=True, stop=True,
                )
                if i % 2 == 0:
                    v.tensor_copy(so[:, i * MM:(i + 1) * MM], aps[:])
                else:
                    s.copy(so[:, i * MM:(i + 1) * MM], aps[:])
            ring = nc.sync if kg % 2 == 0 else nc.gpsimd
            ring.dma_start(ov[n, :, :, k * CH:(k + 1) * CH], so[:])


def make_nc(*, nl=NL, chw=CHW, n_cores=N_CORES):
    import concourse.bacc as bacc
    import concourse.mybir as mybir
    import concourse.tile as tile

    nc = bacc.Bacc(
        "TRN2",
        target_bir_lowering=False,
        debug=False,
        enable_asserts=False,
        num_devices=n_cores,
    )
    x_dr = nc.dram_tensor("x", [nl, G, chw], mybir.dt.float32,
                          kind="ExternalInput")
    out_dr = nc.dram_tensor("out", [nl, G, chw], mybir.dt.float32,
                            kind="ExternalOutput")
    with tile.TileContext(nc) as tc:
        build_graph(nc, tc, x_dr.ap(), out_dr.ap(),
                    nl=nl, chw=chw, n_cores=n_cores)
    nc.compile()
    return nc


def kernel(x: np.ndarray) -> np.ndarray:
    from concourse.bass_utils import run_bass_kernel_spmd

    assert x.shape == (N_FULL, G, C, H, W) and x.dtype == np.float32
    xr = np.ascontiguousarray(x.reshape(N_FULL, G, CHW))
    in_maps = [
        {"x": np.ascontiguousarray(xr[c * NL:(c + 1) * NL])}
        for c in range(N_CORES)
    ]
    nc = make_nc()
    trace = bool(int(os.environ.get("KERNEL_TRACE", "0")))
    res = run_bass_kernel_spmd(
        nc, in_maps, core_ids=list(range(N_CORES)), trace=trace,
    )
    if trace and res.exec_time_ns is not None:
        print(f"HW exec time: {res.exec_time_ns} ns")
    out = np.concatenate([res.results[c]["out"] for c in range(N_CORES)], axis=0)
    return np.ascontiguousarray(out.reshape(N_FULL, G, C, H, W))
